# revision 23
# baseline (speedup 1.0000x reference)
"""Binomial resampling kernel for Trainium2 (8 NeuronCores, SPMD).

y_ng ~ Binomial(n=x_ng, p~U(0.5,0.99)) elementwise.

Algorithm (per element):
  - per-element RNG: two xorshift32 chains (seeded from host), one u32 draw
    each per element-pair per tile; u16 halves give uniforms.
  - p = 0.5 + 0.49*u_p
  - Large-variance branch (npq > SIG0SQ): continuity-corrected, skew-corrected
    (Cornish-Fisher), variance-renormalized Gaussian quantile:
        y = floor(clamp(mu + 0.5 + sig'*z + ((q-p)/6)(z^2-1), 0, n))
    with sig' = sqrt(npq - (q-p)^2/18) so mean AND variance match exactly,
    z from Box-Muller.
  - Small-variance branch: exact inverse-CDF of failures m ~ Bin(n, q)
    truncated at m<=4 (exact for n<=4; tail error < ~2e-3 conditional worst
    case, ~1e-4 after mixing), y = n - m.
Shard rows across 8 cores; each core gets its own RNG seeds.
"""
import sys
import numpy as np

for _p in ("/opt/trn_rl_repo", "/root/.axon_site/_ro/trn_rl_repo"):
    if _p not in sys.path:
        sys.path.insert(0, _p)

import concourse.bass as bass
import concourse.bacc as bacc
import concourse.mybir as mybir
from concourse.tile import TileContext

A = mybir.AluOpType
F = mybir.ActivationFunctionType
DT = mybir.dt

N_CELLS, N_GENES = 4096, 36601
N_CORES = 8
ROWS_PER_CORE = N_CELLS // N_CORES              # 512
NCOLS = ROWS_PER_CORE * N_GENES // 128          # 146404 (flat [128, NCOLS] per core)
T = 2048                                        # columns per tile
HT = T // 2

C16 = 2.0 ** -16
PC = 0.49 * C16          # p-scale per u16 step
P_OFF = 0.5 + PC / 2     # p = p0 + P_OFF,  p0 = u16 * PC
Q_OFF = 0.5 - PC / 2     # q = Q_OFF - p0
SIG0SQ = 0.75            # variance threshold between branches
K_INV = 4                # CDF thresholds in the exact branch (m <= 4)
PI = float(np.pi)


def _stt_int(nc, out, in0, imm, in1, op0, op1):
    """scalar_tensor_tensor with an integer immediate (for bitvec ops)."""
    eng = nc.vector
    return eng.add_instruction(
        mybir.InstTensorScalarPtr(
            name=nc.get_next_instruction_name(),
            is_scalar_tensor_tensor=True,
            op0=op0,
            op1=op1,
            ins=[
                eng.lower_ap(in0),
                mybir.ImmediateValue(dtype=DT.uint32, value=int(imm)),
                eng.lower_ap(in1),
            ],
            outs=[eng.lower_ap(out)],
        )
    )


def _reg_const(nc, v, dtype=DT.float32):
    key = (dtype, float(v))
    if key in nc.const_aps.aps:
        return
    t = nc.alloc_sbuf_tensor(f"const-{dtype.name}-{v}", [128, 1], dtype)
    nc.gpsimd.memset(t.ap(), float(v))
    nc.const_aps.aps[key] = t.ap()


def build_bass(ncols=NCOLS, tile_t=T):
    ht = tile_t // 2
    nc = bacc.Bacc()
    for v in (2.0 ** -17, -2.0, 2 * PI, -PI, -2 * PI, 0.5 * PI, -0.5, P_OFF, Q_OFF,
              C16, PC, -PC / 6, PC / 2, 0.5, 1.0 / 2, 1.0 / 3):
        _reg_const(nc, v)
    nc.all_engine_barrier()

    x_d = nc.dram_tensor("x", [128, ncols], DT.float32, kind="ExternalInput")
    seed_d = nc.dram_tensor("seed", [128, 2 * ht], DT.uint32, kind="ExternalInput")
    y_d = nc.dram_tensor("y", [128, ncols], DT.float32, kind="ExternalOutput")

    n_full, rem = divmod(ncols, tile_t)
    tiles = [tile_t] * n_full + ([rem] if rem else [])

    with TileContext(nc) as tc:
        with (
            tc.tile_pool(name="state", bufs=1) as st_pool,
            tc.tile_pool(name="hp", bufs=1) as hp,
            tc.tile_pool(name="tp", bufs=1) as tp,
            tc.tile_pool(name="io", bufs=2) as io,
        ):
            Sa = st_pool.tile([128, ht], DT.uint32, tag="Sa")
            Sb = st_pool.tile([128, ht], DT.uint32, tag="Sb")
            dummy = st_pool.tile([128, 1], DT.float32, tag="dummy")
            nc.sync.dma_start(out=Sa[:, :], in_=seed_d[:, :ht])
            nc.sync.dma_start(out=Sb[:, :], in_=seed_d[:, ht:])

            off = 0
            for ti, tw in enumerate(tiles):
                h = tw // 2
                sa = Sa[:, :h]
                sb = Sb[:, :h]

                x = io.tile([128, tw], DT.float32, tag="x")
                y = io.tile([128, tw], DT.float32, tag="y")
                nc.gpsimd.dma_start(out=x[:, :], in_=x_d[:, off:off + tw])
                # tiny Pool read of x: lets the issuing engine observe the DMA lane,
                # so the next same-slot load's WAW wait is elided (direct2d DMA
                # supports only one sync wait).
                nc.gpsimd.tensor_copy(dummy[:, :1], x[:, :1])

                # ---- RNG: advance both chains in place (xorshift32) ----
                _stt_int(nc, sa, sa, 13, sa, A.logical_shift_left, A.bitwise_xor)
                _stt_int(nc, sa, sa, 17, sa, A.logical_shift_right, A.bitwise_xor)
                _stt_int(nc, sa, sa, 5, sa, A.logical_shift_left, A.bitwise_xor)
                _stt_int(nc, sb, sb, 13, sb, A.logical_shift_left, A.bitwise_xor)
                _stt_int(nc, sb, sb, 17, sb, A.logical_shift_right, A.bitwise_xor)
                _stt_int(nc, sb, sb, 5, sb, A.logical_shift_left, A.bitwise_xor)

                # ---- extract u16 lattices (bitvec ops must stay u32->u32) ----
                w1_t = hp.tile([128, ht], DT.uint32, tag="w1")
                w2_t = hp.tile([128, ht], DT.uint32, tag="w2")
                u1f_t = hp.tile([128, ht], DT.float32, tag="u1f")
                u2f_t = hp.tile([128, ht], DT.float32, tag="u2f")
                w1 = w1_t[:, :h]
                w2 = w2_t[:, :h]
                u1f = u1f_t[:, :h]   # hi16 * 2^-16
                u2f = u2f_t[:, :h]   # lo16 * 2^-16
                w3_t = hp.tile([128, ht], DT.uint32, tag="w3")
                w4_t = hp.tile([128, ht], DT.uint32, tag="w4")
                w3 = w3_t[:, :h]
                w4 = w4_t[:, :h]
                nc.vector.tensor_scalar(w1, sa, 16, None, A.logical_shift_right)
                nc.vector.tensor_scalar(w2, sa, 65535, None, A.bitwise_and)
                nc.scalar.activation(u1f, w1, F.Copy, bias=0.0, scale=C16)
                nc.scalar.activation(u2f, w2, F.Copy, bias=0.0, scale=C16)
                pv = tp.tile([128, tw], DT.float32, tag="pv")            # p0 = 0.49*u16/2^16
                nc.vector.tensor_scalar(w3, sb, 16, None, A.logical_shift_right)
                nc.vector.tensor_scalar(w4, sb, 65535, None, A.bitwise_and)
                nc.scalar.activation(pv[:, :h], w3, F.Copy, bias=0.0, scale=PC)
                nc.scalar.activation(pv[:, h:], w4, F.Copy, bias=0.0, scale=PC)

                # ---- Box-Muller ----
                lnu_t = hp.tile([128, ht], DT.float32, tag="lnu")
                r_t = hp.tile([128, ht], DT.float32, tag="r")
                s1_t = hp.tile([128, ht], DT.float32, tag="s1")
                s2_t = hp.tile([128, ht], DT.float32, tag="s2")
                lnu = lnu_t[:, :h]
                r = r_t[:, :h]
                s1 = s1_t[:, :h]
                s2 = s2_t[:, :h]
                # trig table-set group: abs + both sins
                tabs_t = hp.tile([128, ht], DT.float32, tag="tabs")
                tabs = tabs_t[:, :h]
                nc.scalar.activation(tabs, u2f, F.Abs, bias=-0.5, scale=1.0)
                nc.scalar.activation(s1, u2f, F.Sin, bias=-PI, scale=2 * PI)
                nc.scalar.activation(s2, tabs, F.Sin, bias=0.5 * PI, scale=-2 * PI)
                # ln/exp table-set group: r = sqrt(-2 lnu) = exp(0.5 ln(-2 lnu))
                nc.scalar.activation(lnu, u1f, F.Ln, bias=2.0 ** -17, scale=1.0)
                nc.scalar.activation(r, lnu, F.Ln, bias=0.0, scale=-2.0)
                nc.scalar.activation(r, r, F.Exp, bias=0.0, scale=0.5)
                z = tp.tile([128, tw], DT.float32, tag="z")
                nc.vector.tensor_mul(z[:, :h], r, s1)
                nc.vector.tensor_mul(z[:, h:], r, s2)

                # ---- moments ----
                e3 = tp.tile([128, tw], DT.float32, tag="e3")     # (q-p)/6 = -(p0+PC/2)/3
                mu = tp.tile([128, tw], DT.float32, tag="mu")
                vneg = tp.tile([128, tw], DT.float32, tag="vneg")  # -npq
                d = tp.tile([128, tw], DT.float32, tag="d")
                nc.scalar.activation(e3[:, :], pv[:, :], F.Copy, bias=-PC / 6, scale=-1.0 / 3.0)
                nc.vector.scalar_tensor_tensor(mu[:, :], pv[:, :], P_OFF, x[:, :], A.add, A.mult)
                nc.vector.scalar_tensor_tensor(vneg[:, :], pv[:, :], Q_OFF, mu[:, :], A.subtract, A.mult)
                # d = (p0 + PC/2)^2 on ACT; arg = v - (2/9) d = (d * -2/9) - vneg
                nc.scalar.activation(d[:, :], pv[:, :], F.Square, bias=PC / 2, scale=1.0)
                nc.vector.scalar_tensor_tensor(d[:, :], d[:, :], -2.0 / 9.0, vneg[:, :], A.mult, A.subtract)
                nc.vector.tensor_scalar(d[:, :], d[:, :], 1e-30, None, A.max)
                sig = tp.tile([128, tw], DT.float32, tag="sig")
                nc.scalar.activation(sig[:, :], d[:, :], F.Ln, bias=0.0, scale=1.0)
                nc.scalar.activation(sig[:, :], sig[:, :], F.Exp, bias=0.0, scale=0.5)

                # ---- Gaussian branch ----
                z2 = tp.tile([128, tw], DT.float32, tag="z2")
                nc.scalar.activation(z2[:, :], z[:, :], F.Square, bias=0.0, scale=1.0)
                nc.vector.scalar_tensor_tensor(z2[:, :], z2[:, :], 1.0, e3[:, :], A.subtract, A.mult)  # sk
                nc.vector.tensor_mul(sig[:, :], sig[:, :], z[:, :])                                    # m1
                nc.vector.scalar_tensor_tensor(sig[:, :], mu[:, :], 0.5, sig[:, :], A.add, A.add)      # mu+.5+m1
                nc.vector.tensor_add(y[:, :], sig[:, :], z2[:, :])                                     # y0
                nc.vector.scalar_tensor_tensor(y[:, :], y[:, :], 0.0, x[:, :], A.max, A.min)           # clamp

                # ---- exact small-variance branch: m ~ Bin(n, q), y = n - m ----
                lnp = tp.tile([128, tw], DT.float32, tag="e3")   # alias: e3 dead
                lnq = tp.tile([128, tw], DT.float32, tag="lnq")
                rr = tp.tile([128, tw], DT.float32, tag="z")     # alias: z dead
                a = tp.tile([128, tw], DT.float32, tag="mu")     # alias: mu dead
                pmf = tp.tile([128, tw], DT.float32, tag="pmf")
                cdf = tp.tile([128, tw], DT.float32, tag="cdf")
                acc = tp.tile([128, tw], DT.float32, tag="acc")
                nc.scalar.activation(lnp[:, :], pv[:, :], F.Ln, bias=P_OFF, scale=1.0)
                nc.scalar.activation(lnq[:, :], pv[:, :], F.Ln, bias=Q_OFF, scale=-1.0)
                nc.vector.tensor_mul(a[:, :], x[:, :], lnp[:, :])
                nc.scalar.activation(pmf[:, :], a[:, :], F.Exp, bias=0.0, scale=1.0)
                nc.vector.tensor_sub(lnq[:, :], lnq[:, :], lnp[:, :])
                nc.scalar.activation(rr[:, :], lnq[:, :], F.Exp, bias=0.0, scale=1.0)
                nc.scalar.activation(cdf[:, :], pmf[:, :], F.Copy, bias=0.0, scale=1.0)
                nc.vector.tensor_tensor(acc[:, :h], u1f, cdf[:, :h], A.is_ge)
                nc.vector.tensor_tensor(acc[:, h:], u1f, cdf[:, h:], A.is_ge)
                for k in range(1, K_INV):
                    rk = d  # reuse
                    nc.scalar.activation(rk[:, :], rr[:, :], F.Copy, bias=0.0, scale=1.0 / k)
                    nc.vector.scalar_tensor_tensor(a[:, :], x[:, :], float(k - 1), pmf[:, :], A.subtract, A.mult)
                    nc.vector.tensor_mul(pmf[:, :], a[:, :], rk[:, :])
                    nc.vector.tensor_add(cdf[:, :], cdf[:, :], pmf[:, :])
                    nc.vector.tensor_tensor(a[:, :h], u1f, cdf[:, :h], A.is_ge)
                    nc.vector.tensor_tensor(a[:, h:], u1f, cdf[:, h:], A.is_ge)
                    nc.vector.tensor_add(acc[:, :], acc[:, :], a[:, :])
                nc.vector.tensor_sub(acc[:, :], x[:, :], acc[:, :])   # y_small
                # mask: v <= SIG0SQ  <=>  vneg >= -SIG0SQ (int mask for CopyPredicated)
                msk = tp.tile([128, tw], DT.int32, tag="msk")
                nc.vector.tensor_scalar(msk[:, :], vneg[:, :], -SIG0SQ, None, A.is_ge)
                nc.vector.copy_predicated(y[:, :], msk[:, :], acc[:, :])
                # floor: HW f32->int32 cast rounds-to-nearest; subtract 0.5 - 2^-10
                # (exactly representable; y-C never ties, unlike 0.49999997
                # which ties at every odd integer y and RNE drops it by 1)
                yi = tp.tile([128, tw], DT.int32, tag="yi")
                nc.vector.tensor_scalar(yi[:, :], y[:, :], 0.4990234375, None, A.subtract)

                nc.gpsimd.dma_start(out=y_d[:, off:off + tw], in_=yi[:, :])
                off += tw

    nc.compile()
    return nc


_SEEDS = None


def _make_seeds():
    global _SEEDS
    if _SEEDS is None:
        ss = np.random.SeedSequence(0x5EEDB10B)
        kids = ss.spawn(N_CORES)
        seeds = []
        for i in range(N_CORES):
            s = kids[i].generate_state(128 * 2 * HT, dtype=np.uint32).reshape(128, 2 * HT)
            s[s == 0] = 0x9E3779B9
            seeds.append(s)
        _SEEDS = seeds
    return _SEEDS


_NC = None


def run(x_ng: np.ndarray, trace: bool = False, **kwargs):
    """Run the SPMD kernel; returns (y, BassKernelResults)."""
    global _NC
    from concourse.bass_utils import run_bass_kernel_spmd

    x = np.ascontiguousarray(np.asarray(x_ng, dtype=np.float32))
    assert x.shape == (N_CELLS, N_GENES)
    shards = x.reshape(N_CORES, 128, NCOLS)
    seeds = _make_seeds()

    if _NC is None:
        _NC = build_bass()

    in_maps = [{"x": shards[i], "seed": seeds[i]} for i in range(N_CORES)]
    try:
        res = run_bass_kernel_spmd(_NC, in_maps, list(range(N_CORES)), trace=trace, **kwargs)
    except ModuleNotFoundError:
        # NTFF profiling hook unavailable in this container; run untraced.
        res = run_bass_kernel_spmd(_NC, in_maps, list(range(N_CORES)), trace=False, **kwargs)
    y = np.stack([res.results[i]["y"] for i in range(N_CORES)])
    return y.reshape(N_CELLS, N_GENES).astype(np.float32), res


def kernel(x_ng: np.ndarray) -> np.ndarray:
    y, _ = run(x_ng, trace=False)
    return y


if __name__ == "__main__":
    rng = np.random.RandomState(0)
    x = rng.randint(0, 100, size=(N_CELLS, N_GENES)).astype(np.float32)
    y = kernel(x)
    print("y stats:", y.mean(), y.std(), y.min(), y.max())
